# revision 11
# baseline (speedup 1.0000x reference)
"""MoE FFN (top-2 of 8 experts) Trainium2 kernel.

Strategy: expert-parallel over 8 NeuronCores. The router (logits -> top-2 ->
softmax gates) runs on host in float64 as part of sharding/dispatch; each core
evaluates two expert segments (a slot-packed split of the 8 experts chosen to
minimize the per-core column count), in a feature-major layout (tokens along
the matmul free dimension, expert weights as the stationary operand). Host
combines the expert outputs per token with the gates.

Loop order keeps the PE stationary operand resident across all column chunks
of a segment (kc-outer / chunk-inner), so LDWEIGHTS is amortized over the
whole segment width instead of being paid per matmul.

Self-contained: no imports from the problem directory.
"""

import os
import sys
import types

import numpy as np
import ml_dtypes

import orjson
import concourse.bass as bass
import concourse.tile as tile
from concourse import mybir
from concourse.bass_utils import run_bass_kernel_spmd
import concourse.bass_utils as _bu

# ---------------------------------------------------------------------------
# Toolchain patch: this container's walrus codegen accepts at most ONE
# sync-wait command per instruction, but Tile attaches every required wait to
# the consuming instruction. Rewrite the BIR JSON at the single choke point
# (Bass.to_json_bytes): move all but one wait of a multi-wait instruction onto
# single-wait NoOps inserted immediately before it on the same engine.
# Per-engine streams preserve block order, so a preceding NoOp-with-wait is
# semantically identical to the wait living on the instruction itself.
# ---------------------------------------------------------------------------
if not getattr(bass.Bass, "_mws_patched", False):
    _orig_to_json_bytes = bass.Bass.to_json_bytes
    _mws_ctr = [0]

    def _split_multiwaits(bir):
        for f in bir.get("functions", []):
            for bb in f.get("blocks", []):
                insts = bb.get("instructions", [])
                if not any(
                    len((i.get("sync_info") or {}).get("on_wait") or []) > 1
                    for i in insts
                ):
                    continue
                out = []
                for ins in insts:
                    si = ins.get("sync_info")
                    waits = (si or {}).get("on_wait") or []
                    if len(waits) > 1:
                        for w in waits[:-1]:
                            _mws_ctr[0] += 1
                            out.append({
                                "debug": ins.get("debug", 0),
                                "engine": ins["engine"],
                                "ins": [],
                                "outs": [],
                                "name": f"MWS-{_mws_ctr[0]}",
                                "opcode": "NoOp",
                                "sync_info": {"on_update": [], "on_wait": [w]},
                                "text_hint": "mwsplit",
                            })
                        si["on_wait"] = [waits[-1]]
                    out.append(ins)
                bb["instructions"] = out
        return bir

    def _patched_to_json_bytes(self):
        return orjson.dumps(_split_multiwaits(orjson.loads(_orig_to_json_bytes(self))))

    bass.Bass.to_json_bytes = _patched_to_json_bytes
    bass.Bass._mws_patched = True

# ---------------------------------------------------------------------------
# Optional NTFF profiling shim: the image's `antenv` package lacks
# `axon_hooks`, so trace=True (or BASS_TRACE=1) would crash on import inside
# run_bass_kernel_spmd. Provide the module and register the ctypes hook.
# ---------------------------------------------------------------------------
if "antenv.axon_hooks" not in sys.modules:
    try:
        _mod = types.ModuleType("antenv.axon_hooks")
        _mod._hook = None
        _mod.set_axon_ntff_profile_hook = lambda h: setattr(_mod, "_hook", h)
        _mod.get_axon_ntff_profile_hook = lambda: _mod._hook
        sys.modules["antenv.axon_hooks"] = _mod
        import antenv as _antenv

        _antenv.axon_hooks = _mod
        from trn_agent_boot.trn_boot import _ntff_profile_via_ctypes

        _hook = _ntff_profile_via_ctypes("/opt/axon/libaxon_pjrt.so")
        if _hook is not None:
            _mod.set_axon_ntff_profile_hook(_hook)
        _bu.upload_artifacts = lambda tmpdir: tmpdir  # no cloud bucket here
    except Exception:
        pass

BF16 = ml_dtypes.bfloat16
N_EMBD = 1024
N_EXPERTS = 8
HIDDEN = 4096
N_CORES = 8
KC = N_EMBD // 128   # 8  contraction chunks for layer 1
MH = HIDDEN // 128   # 32 hidden tiles
CT = N_EMBD // 128   # 8  output tiles for layer 2
NSEG = 2             # expert segments per core

# Results of the most recent run (test harness reads exec_time_ns from here).
LAST_RUN = {}


def _route_host(xf, gate_w):
    """Top-2 routing in float64. Returns (idx[N,2], gates[N,2]) fp32."""
    logits = xf.astype(np.float64) @ gate_w.astype(np.float64)  # [N, E]
    order = np.argsort(-logits, axis=1, kind="stable")
    top2 = order[:, :2]                                          # [N, 2]
    vals = np.take_along_axis(logits, top2, axis=1)              # [N, 2]
    vals = vals - vals.max(axis=1, keepdims=True)
    ex = np.exp(vals)
    gates = ex / ex.sum(axis=1, keepdims=True)
    return top2.astype(np.int64), gates.astype(np.float32)


def _slot_plan(counts):
    """Pick per-core segment capacities (c0 >= c1) and expert->slot packing.

    Slots: 8 seg0 slots of width c0, 8 seg1 slots of width c1 (one of each
    per core). Each expert occupies whole slots (top-k experts use two seg0
    slots, middle experts one seg0 + one seg1, bottom experts two seg1
    slots). k is chosen to minimize c0 + c1.

    Returns (c0, c1, slots0, slots1) where slotsN[i] = expert id of core i's
    segment-N slot.
    """
    order = np.argsort(-np.asarray(counts), kind="stable")
    best = None
    for k in range(0, 5):
        m = N_EXPERTS - 2 * k
        top = [counts[order[i]] for i in range(k)]
        mid = [counts[order[k + i]] for i in range(m)]
        bot = [counts[order[k + m + i]] for i in range(k)]
        c0 = max([(-(-t // 2)) for t in top] + [0])
        for extra in (0, 8, 16, 32):  # let c0 grow a little to shrink c1
            cc0 = c0 + extra
            c1 = max(
                [mx - cc0 for mx in mid] + [(-(-b // 2)) for b in bot] + [0]
            )
            if mid and cc0 == 0:
                continue
            cc0 = max(cc0, c1)  # keep c0 >= c1 for chunk layout stability
            tot = cc0 + c1
            if best is None or tot < best[0]:
                best = (tot, k, cc0, c1)
    _tot, k, c0, c1 = best
    m = N_EXPERTS - 2 * k
    slots0, slots1 = [], []
    # top-k: two seg0 slots each; middle: one seg0 + one seg1; bottom: two seg1
    for i in range(k):
        slots0 += [int(order[i])] * 2
    for i in range(m):
        slots0.append(int(order[k + i]))
        slots1.append(int(order[k + i]))
    for i in range(k):
        slots1 += [int(order[k + m + i])] * 2
    assert len(slots0) == N_CORES and len(slots1) == N_CORES
    return c0, c1, slots0, slots1


def _chunks_for(cap, base_off):
    """Balanced column chunks of <=512 covering [base_off, base_off+cap)."""
    nch = max(1, -(-cap // 512))
    base, rem = divmod(cap, nch)
    out = []
    off = base_off
    for i in range(nch):
        sz = base + (1 if i < rem else 0)
        if sz:
            out.append((off, sz))
        off += sz
    return out


def _build_program(c0, c1, chunks0, chunks1):
    """SPMD Bass program for one core: two expert segments, kc-outer loops."""
    nc = bass.Bass("TRN2", target_bir_lowering=False, debug=False,
                   num_devices=N_CORES)
    f32 = mybir.dt.float32
    bf16 = mybir.dt.bfloat16
    cap = c0 + c1

    xt_d = nc.dram_tensor("xt", [128, KC * cap], bf16, kind="ExternalInput")
    w1_d = nc.dram_tensor("w1t", [NSEG, MH, 128, KC * 128], bf16,
                          kind="ExternalInput")
    w2_d = nc.dram_tensor("w2t", [NSEG, CT, 128, MH * 128], bf16,
                          kind="ExternalInput")
    b1_d = nc.dram_tensor("b1t", [NSEG, 128, MH], f32, kind="ExternalInput")
    b2_d = nc.dram_tensor("b2t", [NSEG, 128, CT], f32, kind="ExternalInput")
    yt_d = nc.dram_tensor("yt", [CT, 128, cap], bf16, kind="ExternalOutput")

    segs = [(0, chunks0), (1, chunks1)]

    with tile.TileContext(nc) as tc:
        with (
            tc.tile_pool(name="big", bufs=1) as big,
            tc.tile_pool(name="w1p", bufs=3) as w1p,
            tc.tile_pool(name="w2p", bufs=2) as w2p,
            tc.tile_pool(name="yp", bufs=4) as yp,
            tc.tile_pool(name="pp", bufs=2, space="PSUM") as pp,
        ):
            xsb = big.tile([128, KC, cap], bf16)
            ht = big.tile([128, MH, cap], bf16)
            warm = big.tile([128, 512], bf16)
            b1sb = big.tile([128, NSEG, MH], f32)
            b2sb = big.tile([128, NSEG, CT], f32)
            nc.sync.dma_start(b1sb[:], b1_d.rearrange("s p m -> p s m"))
            nc.sync.dma_start(b2sb[:], b2_d.rearrange("s p m -> p s m"))

            # X is laid out kc-major over the whole core window in DRAM
            # ([128, KC, cap]), so each transfer below is a large linear
            # read with multi-KB rows. Segment 0 first, in kc-pair pieces so
            # the first kc-outer sweep only waits for its own quarter; the
            # two engine queues (scalar/sync) run ~175 GB/s each and stream
            # in parallel. Segment 1's X is deferred into the mh loop below
            # so it never delays segment 0. gpsimd issues no DMAs at all --
            # a single gpsimd DMA makes the teardown dge_drain ~4.6us.
            h = KC // 2
            xv = xt_d.rearrange("p (k t) -> p k t", k=KC)
            nc.scalar.dma_start(xsb[:, 0:2, 0:c0], xv[:, 0:2, 0:c0])
            nc.scalar.dma_start(xsb[:, 2:4, 0:c0], xv[:, 2:4, 0:c0])

            # PE warm-up: ~18 dummy matmuls on zeroed SBUF keep the tensor
            # engine busy from right after the preamble so the HAM clock
            # gate reaches 8/8 before the first real matmul (otherwise the
            # first ~4us of real work runs at half clock). No DMA deps.
            nc.vector.memset(warm[:], 0)
            wps = pp.tile([128, 512], f32, tag="warm", name="wps")
            for _ in range(18):
                nc.tensor.matmul(wps[:], warm[:, :128], warm[:], start=True,
                                 stop=True)

            # ---- Layer 1: ht[h, t] = gelu(sum_c W1[c, h] * x[c, t] + b1[h])
            # Segment-outer so the PE can start as soon as segment 0's X and
            # first slab land; kc-outer / chunk-inner so one stationary tile
            # serves every chunk of the segment.
            for seg, chs in segs:
                for mh in range(MH):
                    w1sb = w1p.tile([128, KC * 128], bf16, tag="w1s")
                    nc.sync.dma_start(w1sb[:], w1_d[seg, mh])
                    if seg == 0 and mh == 0:
                        nc.sync.dma_start(xsb[:, 4:6, 0:c0], xv[:, 4:6, 0:c0])
                        nc.sync.dma_start(xsb[:, 6:8, 0:c0], xv[:, 6:8, 0:c0])
                    elif seg == 0 and mh == 2:
                        # segment 1's X; not needed for ~100us
                        nc.scalar.dma_start(xsb[:, :h, c0:cap],
                                            xv[:, :h, c0:cap])
                    elif seg == 0 and mh == 4:
                        nc.sync.dma_start(xsb[:, h:, c0:cap],
                                          xv[:, h:, c0:cap])
                    pss = [pp.tile([128, 512], f32, tag=f"ps{i}", name=f"ps{i}")
                           for i in range(len(chs))]
                    for kc in range(KC):
                        wk = w1sb[:, kc * 128:(kc + 1) * 128]
                        for i, (off, sz) in enumerate(chs):
                            nc.tensor.matmul(
                                pss[i][:, :sz],
                                wk,
                                xsb[:, kc, off:off + sz],
                                start=(kc == 0),
                                stop=(kc == KC - 1),
                            )
                    for i, (off, sz) in enumerate(chs):
                        nc.scalar.activation(
                            ht[:, mh, off:off + sz],
                            pss[i][:, :sz],
                            mybir.ActivationFunctionType.Gelu,
                            bias=b1sb[:, seg, mh:mh + 1],
                        )

            # ---- Layer 2: y[c, t] = sum_h W2[h, c] * ht[h, t] + b2[c]
            for seg, chs in segs:
                for ct in range(CT):
                    w2sb = w2p.tile([128, MH * 128], bf16, tag="w2s")
                    nc.sync.dma_start(w2sb[:], w2_d[seg, ct])
                    pss = [pp.tile([128, 512], f32, tag=f"ps{i}", name=f"ps{i}")
                           for i in range(len(chs))]
                    for kh in range(MH):
                        wk = w2sb[:, kh * 128:(kh + 1) * 128]
                        for i, (off, sz) in enumerate(chs):
                            nc.tensor.matmul(
                                pss[i][:, :sz],
                                wk,
                                ht[:, kh, off:off + sz],
                                start=(kh == 0),
                                stop=(kh == MH - 1),
                            )
                    for i, (off, sz) in enumerate(chs):
                        ysb = yp.tile([128, 512], bf16, name="ysb")
                        nc.vector.tensor_scalar_add(ysb[:, :sz], pss[i][:, :sz],
                                                    b2sb[:, seg, ct:ct + 1])
                        yeng = nc.sync if (ct + i) % 2 == 0 else nc.scalar
                        yeng.dma_start(yt_d[ct, :, off:off + sz], ysb[:, :sz])
    return nc


def _prep_weights(w1, b1, w2, b2):
    """Per-expert weight tensors in the kernel's tiled DRAM layouts."""
    w1t = np.ascontiguousarray(
        w1.astype(BF16).reshape(KC, 128, MH, 128).transpose(2, 1, 0, 3)
        .reshape(MH, 128, KC * 128)
    )
    w2t = np.ascontiguousarray(
        w2.astype(BF16).reshape(MH, 128, CT, 128).transpose(2, 1, 0, 3)
        .reshape(CT, 128, MH * 128)
    )
    b1t = np.ascontiguousarray(b1.astype(np.float32).reshape(MH, 128).T)
    b2t = np.ascontiguousarray(b2.astype(np.float32).reshape(CT, 128).T)
    return w1t, w2t, b1t, b2t


def kernel(x, gate_w, w1, b1, w2, b2):
    x = np.asarray(x)
    B, T, C = x.shape
    N = B * T
    xf = np.ascontiguousarray(x.reshape(N, C).astype(np.float32))
    gate_w = np.asarray(gate_w, dtype=np.float32)
    w1 = np.asarray(w1, dtype=np.float32)
    b1 = np.asarray(b1, dtype=np.float32)
    w2 = np.asarray(w2, dtype=np.float32)
    b2 = np.asarray(b2, dtype=np.float32)

    # --- host router + dispatch (the "all-to-all" of the sharding scheme)
    top2, gates = _route_host(xf, gate_w)
    idx_lists = [np.where((top2 == e).any(axis=1))[0] for e in range(N_EXPERTS)]
    counts = [len(ix) for ix in idx_lists]

    c0, c1, slots0, slots1 = _slot_plan(counts)
    cap = c0 + c1
    chunks0 = _chunks_for(c0, 0)
    chunks1 = _chunks_for(c1, c0)

    # Distribute each expert's tokens over its slots (consecutive pieces).
    slot_tokens0 = [None] * N_CORES
    slot_tokens1 = [None] * N_CORES
    for e in range(N_EXPERTS):
        widths, targets = [], []
        for i in range(N_CORES):
            if slots0[i] == e:
                widths.append(c0)
                targets.append((slot_tokens0, i))
            if slots1[i] == e:
                widths.append(c1)
                targets.append((slot_tokens1, i))
        n = counts[e]
        nslots = len(widths)
        # balanced split proportional to slot widths, capped by width
        base = [min(widths[j], n // nslots) for j in range(nslots)]
        rem = n - sum(base)
        j = 0
        while rem > 0:
            take = min(rem, widths[j] - base[j])
            base[j] += take
            rem -= take
            j += 1
        pos = 0
        for j, (arr, i) in enumerate(targets):
            arr[i] = (e, idx_lists[e][pos:pos + base[j]])
            pos += base[j]
        assert pos == n

    # --- per-core inputs
    xf_bf = xf.astype(BF16)
    wprep = [_prep_weights(w1[e], b1[e], w2[e], b2[e]) for e in range(N_EXPERTS)]
    in_maps = []
    core_segs = []  # per core: [(expert, token_idx_array), ...] per segment
    seg_off = [0, c0]
    for c in range(N_CORES):
        segs = [slot_tokens0[c], slot_tokens1[c]]
        core_segs.append(segs)
        xe = np.zeros((cap, C), BF16)
        for seg, (e, ix) in enumerate(segs):
            xe[seg_off[seg]: seg_off[seg] + len(ix)] = xf_bf[ix]
        xt = np.ascontiguousarray(
            xe.reshape(cap, KC, 128).transpose(2, 1, 0).reshape(128, KC * cap))
        in_maps.append({
            "xt": xt,
            "w1t": np.stack([wprep[e][0] for e, _ in segs]),
            "w2t": np.stack([wprep[e][1] for e, _ in segs]),
            "b1t": np.stack([wprep[e][2] for e, _ in segs]),
            "b2t": np.stack([wprep[e][3] for e, _ in segs]),
        })

    # --- build + run
    nc = _build_program(c0, c1, chunks0, chunks1)
    try:
        res = run_bass_kernel_spmd(nc, in_maps, core_ids=list(range(N_CORES)))
    except Exception:
        # transient PJRT/axon execution errors have been observed; retry once
        res = run_bass_kernel_spmd(nc, in_maps, core_ids=list(range(N_CORES)))
    LAST_RUN["exec_time_ns"] = res.exec_time_ns
    LAST_RUN["mean_exec_time_ns"] = res.mean_exec_time_ns
    LAST_RUN["profile_json"] = res.profile_json
    LAST_RUN["results"] = res

    # --- combine (un-dispatch + gate-weighted sum)
    gate_of = np.zeros((N, N_EXPERTS), np.float32)
    gate_of[np.arange(N), top2[:, 0]] = gates[:, 0]
    gate_of[np.arange(N), top2[:, 1]] = gates[:, 1]
    out = np.zeros((N, C), np.float32)
    for c in range(N_CORES):
        yt = np.asarray(res.results[c]["yt"]).astype(np.float32)  # [CT,128,cap]
        yc = yt.transpose(2, 0, 1).reshape(cap, C)                # [cap, C]
        for seg, (e, ix) in enumerate(core_segs[c]):
            ye = yc[seg_off[seg]: seg_off[seg] + len(ix)]
            out[ix] += gate_of[ix, e][:, None] * ye
    return out.reshape(B, T, C).astype(np.float32)
